# revision 3
# baseline (speedup 1.0000x reference)
"""Trainium2 Bass kernel for nn_MatchNet (MLP forward + 60-iter batched PDHG LP solve).

Data-parallel over 8 NeuronCores: batch 2048 -> 256 rows/core. MLP weights and
S are replicated. Each core runs the full unrolled PDHG solve on its shard.

Math (per core, batch rows b=256, n=512 structures, m=64 combos):
    Z = relu(relu(relu(X@W1+b1)@W2+b2)@W3+b3)          (computed in T layout)
    tau = sigma = 0.9/L,  alpha = tau*sigma            (L from host power iter)
    scaled duals p = tau*y1 [m,b]T, q = tau*y2, states e = x-Z, eb = xbar-Z:
      p+  = relu(p + alpha*(S@xbarT - BT))             xbar = Z + eb
      q+  = relu(q - alpha*(Z + eb))
      t1  = p+@S - q+                                  (PSUM)
      d   = e - t1 + tau
      n2  = sum_feat d^2 ; s = relu(1 - tau/max(sqrt(n2),1e-12))
      m_  = s*d ;  e+ = m_ ;  eb+ = 2*m_ - e
    out x = Z + e_final

Layouts: N-folded SBUF tiles [128, 1024]: col b*512+f = (batch 128*bt+p, feat f).
T-folded [128, 1024]: col c*256+j = (feat c*128+p, batch j).
"""

import numpy as np

N_STRUCTS = 512
N_COMBOS = 64
HID = 1024
N_ITERS = 60
N_CORES = 8
B_FULL = 2048
BC = B_FULL // N_CORES  # 256 batch rows per core
NB = BC // 128  # 2 batch sub-tiles
NF = N_STRUCTS // 128  # 4 feature chunks

_BUILD_CACHE = {}


def _power_L(S: np.ndarray) -> float:
    """Replicates reference.py's power iteration for ||K||_2 in float32."""
    S = S.astype(np.float32)
    n = S.shape[1]
    v = np.full((n,), 1.0 / np.sqrt(n), np.float32)
    for _ in range(30):
        v2 = (S.T @ (S @ v) + v).astype(np.float32)
        v = (v2 / np.float32(np.linalg.norm(v2))).astype(np.float32)
    L = np.sqrt(np.vdot(v, (S.T @ (S @ v) + v).astype(np.float32)))
    return float(L)


def _build_nc(tau: float, sigma: float):
    import concourse.bacc as bacc
    import concourse.mybir as mybir
    import concourse.tile as tile

    f32 = mybir.dt.float32
    AF = mybir.ActivationFunctionType
    ALU = mybir.AluOpType
    alpha = tau * sigma

    nc = bacc.Bacc("TRN2", target_bir_lowering=False, debug=False)

    # ---- DRAM I/O (per-core shapes) ----
    d_XT = nc.dram_tensor("xt", [N_COMBOS, BC], f32, kind="ExternalInput")
    d_W1 = nc.dram_tensor("w1", [N_COMBOS, HID], f32, kind="ExternalInput")
    d_b1 = nc.dram_tensor("b1r", [128, 8], f32, kind="ExternalInput")
    d_W2 = nc.dram_tensor("w2", [HID, HID], f32, kind="ExternalInput")
    d_b2 = nc.dram_tensor("b2r", [128, 8], f32, kind="ExternalInput")
    d_W3 = nc.dram_tensor("w3", [HID, N_STRUCTS], f32, kind="ExternalInput")
    d_b3 = nc.dram_tensor("b3r", [128, 4], f32, kind="ExternalInput")
    d_S = nc.dram_tensor("s", [N_COMBOS, N_STRUCTS], f32, kind="ExternalInput")
    d_aST = nc.dram_tensor("ast", [128, 4 * N_COMBOS], f32, kind="ExternalInput")
    d_I = nc.dram_tensor("ident", [128, 128], f32, kind="ExternalInput")
    d_nI = nc.dram_tensor("nident", [128, 128], f32, kind="ExternalInput")
    d_naI = nc.dram_tensor("naident", [128, 128], f32, kind="ExternalInput")
    d_out = nc.dram_tensor("out", [BC, N_STRUCTS], f32, kind="ExternalOutput")

    with tile.TileContext(nc) as tc:
        import contextlib

        stack = contextlib.ExitStack()
        with stack:
            cpool = stack.enter_context(tc.tile_pool(name="consts", bufs=1))

            def cload(dram, shape, tag):
                t = cpool.tile(shape, f32, tag=tag)
                nc.sync.dma_start(t[:], dram.ap())
                return t

            XT = cload(d_XT, [N_COMBOS, BC], "xt")
            W1 = cload(d_W1, [N_COMBOS, HID], "w1")
            b1r = cload(d_b1, [128, 8], "b1r")
            b2r = cload(d_b2, [128, 8], "b2r")
            b3r = cload(d_b3, [128, 4], "b3r")
            S_sb = cload(d_S, [N_COMBOS, N_STRUCTS], "s")
            aST = cload(d_aST, [128, 4 * N_COMBOS], "ast")
            I128 = cload(d_I, [128, 128], "ident")
            nI128 = cload(d_nI, [128, 128], "nident")
            naI128 = cload(d_naI, [128, 128], "naident")
            W2 = []
            for k in range(8):
                t = cpool.tile([128, HID], f32, tag=f"w2_{k}")
                nc.sync.dma_start(t[:], d_W2.ap()[k * 128 : (k + 1) * 128, :])
                W2.append(t)
            W3 = []
            for k in range(8):
                t = cpool.tile([128, N_STRUCTS], f32, tag=f"w3_{k}")
                nc.sync.dma_start(t[:], d_W3.ap()[k * 128 : (k + 1) * 128, :])
                W3.append(t)

            # ---- MLP forward, T layout ----
            zt = []  # final Z^T tiles [128, BC] x4
            with (
                tc.tile_pool(name="mlp_sb", bufs=1) as mpool,
                tc.tile_pool(name="mlp_ps", bufs=4, space="PSUM") as mpsum,
            ):
                z1t = []
                for t in range(8):
                    ps = mpsum.tile([128, BC], f32, tag="mm")
                    nc.tensor.matmul(
                        ps[:], W1[:, t * 128 : (t + 1) * 128], XT[:], start=True, stop=True
                    )
                    sb = mpool.tile([128, BC], f32, tag=f"z1_{t}")
                    nc.scalar.activation(sb[:], ps[:], AF.Relu, bias=b1r[:, t : t + 1])
                    z1t.append(sb)
                z2t = []
                for t in range(8):
                    ps = mpsum.tile([128, BC], f32, tag="mm")
                    for k in range(8):
                        nc.tensor.matmul(
                            ps[:],
                            W2[k][:, t * 128 : (t + 1) * 128],
                            z1t[k][:],
                            start=(k == 0),
                            stop=(k == 7),
                        )
                    sb = mpool.tile([128, BC], f32, tag=f"z2_{t}")
                    nc.scalar.activation(sb[:], ps[:], AF.Relu, bias=b2r[:, t : t + 1])
                    z2t.append(sb)
                for c in range(NF):
                    ps = mpsum.tile([128, BC], f32, tag="mm")
                    for k in range(8):
                        nc.tensor.matmul(
                            ps[:],
                            W3[k][:, c * 128 : (c + 1) * 128],
                            z2t[k][:],
                            start=(k == 0),
                            stop=(k == 7),
                        )
                    sb = cpool.tile([128, BC], f32, tag=f"zt_{c}")
                    nc.scalar.activation(sb[:], ps[:], AF.Relu, bias=b3r[:, c : c + 1])
                    zt.append(sb)

            # ---- PDHG setup ----
            spool = stack.enter_context(tc.tile_pool(name="setup", bufs=1))
            ppool = stack.enter_context(tc.tile_pool(name="pd_ps", bufs=1, space="PSUM"))

            # cSZB = alpha*S@Z^T - alpha*B^T   [64, BC]
            ps = ppool.tile([N_COMBOS, BC], f32, tag="py1")
            for c in range(NF):
                nc.tensor.matmul(
                    ps[:], aST[:, c * 64 : (c + 1) * 64], zt[c][:],
                    start=(c == 0), stop=False,
                )
            nc.tensor.matmul(ps[:], naI128[:64, :64], XT[:], start=False, stop=True)
            cSZB = spool.tile([N_COMBOS, BC], f32, tag="cszb")
            nc.scalar.activation(cSZB[:], ps[:], AF.Copy)

            # Z in N-folded layout via PE transposes
            psz = ppool.tile([128, NB * N_STRUCTS], f32, tag="pT")
            for b in range(NB):
                for c in range(NF):
                    nc.tensor.transpose(
                        psz[:, b * 512 + c * 128 : b * 512 + (c + 1) * 128],
                        zt[c][:, b * 128 : (b + 1) * 128],
                        I128[:],
                    )
            Z = spool.tile([128, NB * N_STRUCTS], f32, tag="zn")
            nc.scalar.activation(Z[:], psz[:], AF.Copy)
            naZ = spool.tile([128, NB * N_STRUCTS], f32, tag="naz")
            nc.scalar.activation(naZ[:], Z[:], AF.Copy, scale=-alpha)

            # ---- PDHG state pools ----
            em_pool = stack.enter_context(tc.tile_pool(name="em", bufs=3))
            eb_pool = stack.enter_context(tc.tile_pool(name="eb", bufs=2))
            p_pool = stack.enter_context(tc.tile_pool(name="pp", bufs=2))
            q_pool = stack.enter_context(tc.tile_pool(name="qq", bufs=2))
            sc_pool = stack.enter_context(tc.tile_pool(name="scratch", bufs=2))

            W = NB * N_STRUCTS  # 1024 folded width

            e = em_pool.tile([128, W], f32, tag="em")
            nc.scalar.activation(e[:], Z[:], AF.Copy, scale=-1.0)
            eb = eb_pool.tile([128, W], f32, tag="eb")
            nc.vector.tensor_copy(eb[:], e[:])
            p = p_pool.tile([N_COMBOS, BC], f32, tag="p")
            nc.vector.memset(p[:], 0.0)
            q = q_pool.tile([128, W], f32, tag="q")
            nc.vector.memset(q[:], 0.0)

            for it in range(N_ITERS):
                # 1) ebT = transpose(eb), T-folded
                psT = ppool.tile([128, W], f32, tag="pT")
                for c in range(NF):
                    for b in range(NB):
                        nc.tensor.transpose(
                            psT[:, c * BC + b * 128 : c * BC + (b + 1) * 128],
                            eb[:, b * 512 + c * 128 : b * 512 + (c + 1) * 128],
                            I128[:],
                        )
                ebT = sc_pool.tile([128, W], f32, tag="ebT")
                nc.scalar.activation(ebT[:], psT[:], AF.Copy)

                # 2) p+ = relu(p + alpha*S@ebT + cSZB)
                ps1 = ppool.tile([N_COMBOS, BC], f32, tag="py1")
                for c in range(NF):
                    nc.tensor.matmul(
                        ps1[:],
                        aST[:, c * 64 : (c + 1) * 64],
                        ebT[:, c * BC : (c + 1) * BC],
                        start=(c == 0), stop=False,
                    )
                nc.tensor.matmul(ps1[:], I128[:64, :64], p[:], start=False, stop=False)
                nc.tensor.matmul(ps1[:], I128[:64, :64], cSZB[:], start=False, stop=True)
                p_new = p_pool.tile([N_COMBOS, BC], f32, tag="p")
                nc.scalar.activation(p_new[:], ps1[:], AF.Relu)

                # 3) q+ = relu(q - alpha*eb - alpha*Z)
                ps2 = ppool.tile([128, W], f32, tag="py2")
                for b in range(NB):
                    sl = slice(b * 512, (b + 1) * 512)
                    nc.tensor.matmul(ps2[:, sl], I128[:], q[:, sl], start=True, stop=False)
                    nc.tensor.matmul(ps2[:, sl], naI128[:], eb[:, sl], start=False, stop=False)
                    nc.tensor.matmul(ps2[:, sl], I128[:], naZ[:, sl], start=False, stop=True)
                q_new = q_pool.tile([128, W], f32, tag="q")
                nc.scalar.activation(q_new[:], ps2[:], AF.Relu)

                # 4) t1 = p+@S - q+   (PSUM)
                ps3 = ppool.tile([128, W], f32, tag="pt1")
                for b in range(NB):
                    sl = slice(b * 512, (b + 1) * 512)
                    nc.tensor.matmul(
                        ps3[:, sl],
                        p_new[:, b * 128 : (b + 1) * 128],
                        S_sb[:],
                        start=True, stop=False,
                    )
                    nc.tensor.matmul(ps3[:, sl], nI128[:], q_new[:, sl], start=False, stop=True)

                # 5) d = (t1*-1 + tau) + e
                d = sc_pool.tile([128, W], f32, tag="d")
                nc.vector.affine_then_add(d[:], ps3[:], e[:], scale=-1.0, bias=tau)

                # 6) n2[b] = sum_f d^2 (ACT square + accumulate)
                dsq = sc_pool.tile([128, W], f32, tag="dsq")
                n2 = sc_pool.tile([128, NB], f32, tag="n2")
                for b in range(NB):
                    sl = slice(b * 512, (b + 1) * 512)
                    nc.scalar.activation(
                        dsq[:, sl], d[:, sl], AF.Square, accum_out=n2[:, b : b + 1]
                    )

                # 7) s = relu(1 - tau/max(sqrt(n2),1e-12))
                nmax = sc_pool.tile([128, NB], f32, tag="nmax")
                nc.vector.tensor_scalar_max(nmax[:], n2[:], 1e-24)
                nrm = sc_pool.tile([128, NB], f32, tag="nrm")
                nc.scalar.activation(nrm[:], nmax[:], AF.Sqrt)
                rr = sc_pool.tile([128, NB], f32, tag="rr")
                rs = sc_pool.tile([128, NB], f32, tag="rs")
                nc.vector.reciprocal_approx_accurate(rr[:], nrm[:], rs[:])
                s = sc_pool.tile([128, NB], f32, tag="s")
                nc.scalar.activation(s[:], rr[:], AF.Relu, bias=1.0, scale=-tau)

                # 8) m = s (.) d  -> becomes e+
                m_ = em_pool.tile([128, W], f32, tag="em")
                for b in range(NB):
                    sl = slice(b * 512, (b + 1) * 512)
                    nc.vector.tensor_scalar_mul(m_[:, sl], d[:, sl], s[:, b : b + 1])

                # 9) eb+ = 2m - e
                eb_new = eb_pool.tile([128, W], f32, tag="eb")
                nc.vector.ln_bwd_dx(
                    eb_new[:], m_[:], e[:], mean_dyx=0.5, mean_dy=0.0, scale=2.0
                )

                e, eb, p, q = m_, eb_new, p_new, q_new

            # ---- output: x = Z + e ----
            xout = sc_pool.tile([128, W], f32, tag="xout")
            nc.vector.tensor_add(xout[:], Z[:], e[:])
            for b in range(NB):
                nc.sync.dma_start(
                    d_out.ap()[b * 128 : (b + 1) * 128, :],
                    xout[:, b * 512 : (b + 1) * 512],
                )

    nc.finalize()
    return nc


def _get_nc(S: np.ndarray):
    key = hash(S.tobytes())
    if key not in _BUILD_CACHE:
        L = _power_L(S)
        tau = 0.9 / L
        sigma = 0.9 / L
        _BUILD_CACHE[key] = (_build_nc(tau, sigma), tau, sigma)
    return _BUILD_CACHE[key]


def _make_in_maps(X, W1, b1, W2, b2, W3, b3, S, tau, sigma):
    alpha = np.float32(tau * sigma)
    Xflat = np.ascontiguousarray(X.reshape(B_FULL, N_COMBOS)).astype(np.float32)
    # aST packed: alpha * S.T chunks [128, 64] side by side -> [128, 256]
    aST_full = (alpha * S.T).astype(np.float32)  # [512, 64]
    aST = np.concatenate(
        [aST_full[c * 128 : (c + 1) * 128, :] for c in range(NF)], axis=1
    )
    aST = np.ascontiguousarray(aST)
    b1r = np.ascontiguousarray(b1.reshape(8, 128).T).astype(np.float32)
    b2r = np.ascontiguousarray(b2.reshape(8, 128).T).astype(np.float32)
    b3r = np.ascontiguousarray(b3.reshape(4, 128).T).astype(np.float32)
    I128 = np.eye(128, dtype=np.float32)
    nI128 = (-np.eye(128)).astype(np.float32)
    naI128 = (-alpha * np.eye(128)).astype(np.float32)
    shared = {
        "w1": np.ascontiguousarray(W1.astype(np.float32)),
        "b1r": b1r,
        "w2": np.ascontiguousarray(W2.astype(np.float32)),
        "b2r": b2r,
        "w3": np.ascontiguousarray(W3.astype(np.float32)),
        "b3r": b3r,
        "s": np.ascontiguousarray(S.astype(np.float32)),
        "ast": aST,
        "ident": I128,
        "nident": nI128,
        "naident": naI128,
    }
    in_maps = []
    for c in range(N_CORES):
        xt = np.ascontiguousarray(Xflat[c * BC : (c + 1) * BC, :].T)
        in_maps.append({**shared, "xt": xt})
    return in_maps


def kernel(X, W1, b1, W2, b2, W3, b3, S, batch_size):
    from concourse.bass_utils import run_bass_kernel_spmd

    X = np.asarray(X)
    S = np.asarray(S)
    nc, tau, sigma = _get_nc(np.ascontiguousarray(S.astype(np.float32)))
    in_maps = _make_in_maps(
        X,
        np.asarray(W1),
        np.asarray(b1),
        np.asarray(W2),
        np.asarray(b2),
        np.asarray(W3),
        np.asarray(b3),
        S,
        tau,
        sigma,
    )
    res = run_bass_kernel_spmd(nc, in_maps, core_ids=list(range(N_CORES)))
    out = np.concatenate([res.results[c]["out"] for c in range(N_CORES)], axis=0)
    return out.astype(np.float32)


# revision 5
# speedup vs baseline: 1.2614x; 1.2614x over previous
"""Trainium2 Bass kernel for nn_MatchNet (MLP forward + 60-iter batched PDHG LP solve).

Data-parallel over 8 NeuronCores: batch 2048 -> 256 rows/core. MLP weights and
S are replicated. Each core runs the full unrolled PDHG solve on its shard.

Math (per core, batch rows b=256, n=512 structures, m=64 combos):
    Z = relu(relu(relu(X@W1+b1)@W2+b2)@W3+b3)          (computed in T layout)
    tau = sigma = 0.9/L,  alpha = tau*sigma            (L from host power iter)
    scaled duals p = tau*y1 [m,b]T, q = tau*y2, states e = x-Z, eb = xbar-Z:
      p+  = relu(p + alpha*(S@xbarT - BT))             xbar = Z + eb
      q+  = relu(q - alpha*(Z + eb))
      t1  = p+@S - q+                                  (PSUM)
      d   = e - t1 + tau
      n2  = sum_feat d^2 ; s = relu(1 - tau/max(sqrt(n2),1e-12))
      m_  = s*d ;  e+ = m_ ;  eb+ = 2*m_ - e
    out x = Z + e_final

Layouts: N-folded SBUF tiles [128, 1024]: col b*512+f = (batch 128*bt+p, feat f).
T-folded [128, 1024]: col c*256+j = (feat c*128+p, batch j).
"""

import numpy as np

N_STRUCTS = 512
N_COMBOS = 64
HID = 1024
N_ITERS = 60
N_CORES = 8
B_FULL = 2048
BC = B_FULL // N_CORES  # 256 batch rows per core
NB = BC // 128  # 2 batch sub-tiles
NF = N_STRUCTS // 128  # 4 feature chunks

_BUILD_CACHE = {}


def _power_L(S: np.ndarray) -> float:
    """Replicates reference.py's power iteration for ||K||_2 in float32."""
    S = S.astype(np.float32)
    n = S.shape[1]
    v = np.full((n,), 1.0 / np.sqrt(n), np.float32)
    for _ in range(30):
        v2 = (S.T @ (S @ v) + v).astype(np.float32)
        v = (v2 / np.float32(np.linalg.norm(v2))).astype(np.float32)
    L = np.sqrt(np.vdot(v, (S.T @ (S @ v) + v).astype(np.float32)))
    return float(L)


def _build_nc(tau: float, sigma: float):
    import concourse.bacc as bacc
    import concourse.mybir as mybir
    import concourse.tile as tile

    f32 = mybir.dt.float32
    AF = mybir.ActivationFunctionType
    ALU = mybir.AluOpType
    alpha = tau * sigma

    nc = bacc.Bacc("TRN2", target_bir_lowering=False, debug=False)

    # ---- DRAM I/O (per-core shapes) ----
    d_XT = nc.dram_tensor("xt", [N_COMBOS, BC], f32, kind="ExternalInput")
    d_W1 = nc.dram_tensor("w1", [N_COMBOS, HID], f32, kind="ExternalInput")
    d_b1 = nc.dram_tensor("b1r", [128, 8], f32, kind="ExternalInput")
    d_W2 = nc.dram_tensor("w2", [HID, HID], f32, kind="ExternalInput")
    d_b2 = nc.dram_tensor("b2r", [128, 8], f32, kind="ExternalInput")
    d_W3 = nc.dram_tensor("w3", [HID, N_STRUCTS], f32, kind="ExternalInput")
    d_b3 = nc.dram_tensor("b3r", [128, 4], f32, kind="ExternalInput")
    d_S = nc.dram_tensor("s", [N_COMBOS, N_STRUCTS], f32, kind="ExternalInput")
    d_aST = nc.dram_tensor("ast", [128, 4 * N_COMBOS], f32, kind="ExternalInput")
    d_I = nc.dram_tensor("ident", [128, 128], f32, kind="ExternalInput")
    d_nI = nc.dram_tensor("nident", [128, 128], f32, kind="ExternalInput")
    d_naI = nc.dram_tensor("naident", [128, 128], f32, kind="ExternalInput")
    d_out = nc.dram_tensor("out", [BC, N_STRUCTS], f32, kind="ExternalOutput")

    with tile.TileContext(nc) as tc:
        import contextlib

        stack = contextlib.ExitStack()
        with stack:
            cpool = stack.enter_context(tc.tile_pool(name="consts", bufs=1))

            def cload(dram, shape, tag):
                t = cpool.tile(shape, f32, tag=tag)
                nc.sync.dma_start(t[:], dram.ap())
                return t

            XT = cload(d_XT, [N_COMBOS, BC], "xt")
            W1 = cload(d_W1, [N_COMBOS, HID], "w1")
            b1r = cload(d_b1, [128, 8], "b1r")
            b2r = cload(d_b2, [128, 8], "b2r")
            b3r = cload(d_b3, [128, 4], "b3r")
            S_sb = cload(d_S, [N_COMBOS, N_STRUCTS], "s")
            aST = cload(d_aST, [128, 4 * N_COMBOS], "ast")
            I128 = cload(d_I, [128, 128], "ident")
            nI128 = cload(d_nI, [128, 128], "nident")
            naI128 = cload(d_naI, [128, 128], "naident")
            W2 = []
            for k in range(8):
                t = cpool.tile([128, HID], f32, tag=f"w2_{k}")
                nc.sync.dma_start(t[:], d_W2.ap()[k * 128 : (k + 1) * 128, :])
                W2.append(t)
            W3 = []
            for k in range(8):
                t = cpool.tile([128, N_STRUCTS], f32, tag=f"w3_{k}")
                nc.sync.dma_start(t[:], d_W3.ap()[k * 128 : (k + 1) * 128, :])
                W3.append(t)

            # ---- MLP forward, T layout ----
            zt = []  # final Z^T tiles [128, BC] x4
            with (
                tc.tile_pool(name="mlp_sb", bufs=1) as mpool,
                tc.tile_pool(name="mlp_ps", bufs=4, space="PSUM") as mpsum,
            ):
                z1t = []
                for t in range(8):
                    ps = mpsum.tile([128, BC], f32, tag="mm")
                    nc.tensor.matmul(
                        ps[:], W1[:, t * 128 : (t + 1) * 128], XT[:], start=True, stop=True
                    )
                    sb = mpool.tile([128, BC], f32, tag=f"z1_{t}")
                    nc.scalar.activation(sb[:], ps[:], AF.Relu, bias=b1r[:, t : t + 1])
                    z1t.append(sb)
                z2t = []
                for t in range(8):
                    ps = mpsum.tile([128, BC], f32, tag="mm")
                    for k in range(8):
                        nc.tensor.matmul(
                            ps[:],
                            W2[k][:, t * 128 : (t + 1) * 128],
                            z1t[k][:],
                            start=(k == 0),
                            stop=(k == 7),
                        )
                    sb = mpool.tile([128, BC], f32, tag=f"z2_{t}")
                    nc.scalar.activation(sb[:], ps[:], AF.Relu, bias=b2r[:, t : t + 1])
                    z2t.append(sb)
                for c in range(NF):
                    ps = mpsum.tile([128, BC], f32, tag="mm")
                    for k in range(8):
                        nc.tensor.matmul(
                            ps[:],
                            W3[k][:, c * 128 : (c + 1) * 128],
                            z2t[k][:],
                            start=(k == 0),
                            stop=(k == 7),
                        )
                    sb = cpool.tile([128, BC], f32, tag=f"zt_{c}")
                    nc.scalar.activation(sb[:], ps[:], AF.Relu, bias=b3r[:, c : c + 1])
                    zt.append(sb)

            # ---- PDHG setup ----
            spool = stack.enter_context(tc.tile_pool(name="setup", bufs=1))
            setup_ps = contextlib.ExitStack()
            ppool = setup_ps.enter_context(tc.tile_pool(name="pd_ps", bufs=1, space="PSUM"))

            # cSZB = alpha*S@Z^T - alpha*B^T   [64, BC]
            ps = ppool.tile([N_COMBOS, BC], f32, tag="py1")
            for c in range(NF):
                nc.tensor.matmul(
                    ps[:], aST[:, c * 64 : (c + 1) * 64], zt[c][:],
                    start=(c == 0), stop=False,
                )
            nc.tensor.matmul(ps[:], naI128[:64, :64], XT[:], start=False, stop=True)
            cSZB = spool.tile([N_COMBOS, BC], f32, tag="cszb")
            nc.scalar.activation(cSZB[:], ps[:], AF.Copy)

            # Z in N-folded layout via PE transposes
            psz = ppool.tile([128, NB * N_STRUCTS], f32, tag="pT")
            for b in range(NB):
                for c in range(NF):
                    nc.tensor.transpose(
                        psz[:, b * 512 + c * 128 : b * 512 + (c + 1) * 128],
                        zt[c][:, b * 128 : (b + 1) * 128],
                        I128[:],
                    )
            Z = spool.tile([128, NB * N_STRUCTS], f32, tag="zn")
            nc.scalar.activation(Z[:], psz[:], AF.Copy)
            naZ = spool.tile([128, NB * N_STRUCTS], f32, tag="naz")
            nc.scalar.activation(naZ[:], Z[:], AF.Copy, scale=-alpha)
            setup_ps.close()

            # ---- PDHG state pools ----
            em_pool = stack.enter_context(tc.tile_pool(name="em", bufs=3))
            eb_pool = stack.enter_context(tc.tile_pool(name="eb", bufs=2))
            p_pool = stack.enter_context(tc.tile_pool(name="pp", bufs=2))
            q_pool = stack.enter_context(tc.tile_pool(name="qq", bufs=2))
            sc_pool = stack.enter_context(tc.tile_pool(name="scratch", bufs=2))
            ppool_T = stack.enter_context(tc.tile_pool(name="ps_T", bufs=1, space="PSUM"))
            ppool_y1 = stack.enter_context(tc.tile_pool(name="ps_y1", bufs=2, space="PSUM"))
            ppool_t1 = stack.enter_context(tc.tile_pool(name="ps_t1", bufs=2, space="PSUM"))

            W = NB * N_STRUCTS  # 1024 folded width

            e = em_pool.tile([128, W], f32, tag="em")
            nc.scalar.activation(e[:], Z[:], AF.Copy, scale=-1.0)
            eb = eb_pool.tile([128, W], f32, tag="eb")
            nc.vector.tensor_copy(eb[:], e[:])
            # pc = p + cSZB (state); p0 = 0
            pc = p_pool.tile([N_COMBOS, BC], f32, tag="pc")
            nc.vector.tensor_copy(pc[:], cSZB[:])
            q = q_pool.tile([128, W], f32, tag="q")
            nc.vector.memset(q[:], 0.0)

            for it in range(N_ITERS):
                # 1) ebT = transpose(eb), T-folded [128, c*256 + b*128 + j]
                psT = ppool_T.tile([128, W], f32, tag="pT")
                for c in range(NF):
                    for b in range(NB):
                        nc.tensor.transpose(
                            psT[:, c * BC + b * 128 : c * BC + (b + 1) * 128],
                            eb[:, b * 512 + c * 128 : b * 512 + (c + 1) * 128],
                            I128[:],
                        )
                ebT = sc_pool.tile([128, W], f32, tag="ebT")
                for h in range(2):
                    sl = slice(h * 512, (h + 1) * 512)
                    nc.scalar.activation(ebT[:, sl], psT[:, sl], AF.Copy)

                # 2) p+ = relu(pc + alpha*S@ebT); pc+ = p+ + cSZB
                ps1 = ppool_y1.tile([N_COMBOS, BC], f32, tag="py1")
                nc.tensor.matmul(ps1[:], I128[:64, :64], pc[:], start=True, stop=False)
                for c in range(NF):
                    nc.tensor.matmul(
                        ps1[:],
                        aST[:, c * 64 : (c + 1) * 64],
                        ebT[:, c * BC : (c + 1) * BC],
                        start=False, stop=(c == NF - 1),
                    )
                p_new = p_pool.tile([N_COMBOS, BC], f32, tag="p")
                nc.scalar.activation(p_new[:], ps1[:], AF.Relu)
                pc_new = p_pool.tile([N_COMBOS, BC], f32, tag="pc")
                nc.vector.tensor_add(pc_new[:], p_new[:], cSZB[:])

                # 3) q+ = relu((q - alpha*Z) - alpha*eb)   [no PE]
                w_ = sc_pool.tile([128, W], f32, tag="w")
                nc.gpsimd.tensor_add(w_[:], q[:], naZ[:])
                h_ = sc_pool.tile([128, W], f32, tag="h")
                nc.vector.scalar_tensor_tensor(
                    h_[:], eb[:], -alpha, w_[:], op0=ALU.mult, op1=ALU.add
                )
                q_new = q_pool.tile([128, W], f32, tag="q")
                nc.scalar.activation(q_new[:], h_[:], AF.Relu)

                # 4) P1 = p+@S (PSUM); u = (e + tau) + q+  [GPSIMD]
                ps3 = ppool_t1.tile([128, W], f32, tag="pt1")
                for b in range(NB):
                    sl = slice(b * 512, (b + 1) * 512)
                    nc.tensor.matmul(
                        ps3[:, sl],
                        p_new[:, b * 128 : (b + 1) * 128],
                        S_sb[:],
                        start=True, stop=True,
                    )
                u = sc_pool.tile([128, W], f32, tag="u")
                nc.gpsimd.scalar_tensor_tensor(
                    u[:], e[:], tau, q_new[:], op0=ALU.add, op1=ALU.add
                )

                # 5) d = -P1 + u
                d = sc_pool.tile([128, W], f32, tag="d")
                nc.vector.scalar_tensor_tensor(
                    d[:], ps3[:], -1.0, u[:], op0=ALU.mult, op1=ALU.add
                )

                # 6) n2[b] = sum_f d^2 (ACT square + accumulate)
                dsq = sc_pool.tile([128, W], f32, tag="dsq")
                n2 = sc_pool.tile([128, NB], f32, tag="n2")
                for b in range(NB):
                    sl = slice(b * 512, (b + 1) * 512)
                    nc.scalar.activation(
                        dsq[:, sl], d[:, sl], AF.Square, accum_out=n2[:, b : b + 1]
                    )

                # 7) s = relu(1 - tau/max(sqrt(n2),1e-12))
                nmax = sc_pool.tile([128, NB], f32, tag="nmax")
                nc.vector.tensor_scalar_max(nmax[:], n2[:], 1e-24)
                nrm = sc_pool.tile([128, NB], f32, tag="nrm")
                nc.scalar.activation(nrm[:], nmax[:], AF.Sqrt)
                rr = sc_pool.tile([128, NB], f32, tag="rr")
                rs = sc_pool.tile([128, NB], f32, tag="rs")
                nc.vector.reciprocal_approx_accurate(rr[:], nrm[:], rs[:])
                s = sc_pool.tile([128, NB], f32, tag="s")
                nc.scalar.activation(s[:], rr[:], AF.Relu, bias=1.0, scale=-tau)

                # 8) m = s (.) d  -> e+   (split DVE / ACT)
                m_ = em_pool.tile([128, W], f32, tag="em")
                nc.vector.tensor_scalar_mul(m_[:, 0:512], d[:, 0:512], s[:, 0:1])
                nc.scalar.activation(
                    m_[:, 512:1024], d[:, 512:1024], AF.Copy, scale=s[:, 1:2]
                )

                # 9) eb+ = 2m - e
                eb_new = eb_pool.tile([128, W], f32, tag="eb")
                nc.vector.ln_bwd_dx(
                    eb_new[:], m_[:], e[:], mean_dyx=0.5, mean_dy=0.0, scale=2.0
                )

                e, eb, q, pc = m_, eb_new, q_new, pc_new

            # ---- output: x = Z + e ----
            xout = sc_pool.tile([128, W], f32, tag="xout")
            nc.vector.tensor_add(xout[:], Z[:], e[:])
            for b in range(NB):
                nc.sync.dma_start(
                    d_out.ap()[b * 128 : (b + 1) * 128, :],
                    xout[:, b * 512 : (b + 1) * 512],
                )

    nc.finalize()
    return nc


def _get_nc(S: np.ndarray):
    key = hash(S.tobytes())
    if key not in _BUILD_CACHE:
        L = _power_L(S)
        tau = 0.9 / L
        sigma = 0.9 / L
        _BUILD_CACHE[key] = (_build_nc(tau, sigma), tau, sigma)
    return _BUILD_CACHE[key]


def _make_in_maps(X, W1, b1, W2, b2, W3, b3, S, tau, sigma):
    alpha = np.float32(tau * sigma)
    Xflat = np.ascontiguousarray(X.reshape(B_FULL, N_COMBOS)).astype(np.float32)
    # aST packed: alpha * S.T chunks [128, 64] side by side -> [128, 256]
    aST_full = (alpha * S.T).astype(np.float32)  # [512, 64]
    aST = np.concatenate(
        [aST_full[c * 128 : (c + 1) * 128, :] for c in range(NF)], axis=1
    )
    aST = np.ascontiguousarray(aST)
    b1r = np.ascontiguousarray(b1.reshape(8, 128).T).astype(np.float32)
    b2r = np.ascontiguousarray(b2.reshape(8, 128).T).astype(np.float32)
    b3r = np.ascontiguousarray(b3.reshape(4, 128).T).astype(np.float32)
    I128 = np.eye(128, dtype=np.float32)
    nI128 = (-np.eye(128)).astype(np.float32)
    naI128 = (-alpha * np.eye(128)).astype(np.float32)
    shared = {
        "w1": np.ascontiguousarray(W1.astype(np.float32)),
        "b1r": b1r,
        "w2": np.ascontiguousarray(W2.astype(np.float32)),
        "b2r": b2r,
        "w3": np.ascontiguousarray(W3.astype(np.float32)),
        "b3r": b3r,
        "s": np.ascontiguousarray(S.astype(np.float32)),
        "ast": aST,
        "ident": I128,
        "nident": nI128,
        "naident": naI128,
    }
    in_maps = []
    for c in range(N_CORES):
        xt = np.ascontiguousarray(Xflat[c * BC : (c + 1) * BC, :].T)
        in_maps.append({**shared, "xt": xt})
    return in_maps


def kernel(X, W1, b1, W2, b2, W3, b3, S, batch_size):
    from concourse.bass_utils import run_bass_kernel_spmd

    X = np.asarray(X)
    S = np.asarray(S)
    nc, tau, sigma = _get_nc(np.ascontiguousarray(S.astype(np.float32)))
    in_maps = _make_in_maps(
        X,
        np.asarray(W1),
        np.asarray(b1),
        np.asarray(W2),
        np.asarray(b2),
        np.asarray(W3),
        np.asarray(b3),
        S,
        tau,
        sigma,
    )
    res = run_bass_kernel_spmd(nc, in_maps, core_ids=list(range(N_CORES)))
    out = np.concatenate([res.results[c]["out"] for c in range(N_CORES)], axis=0)
    return out.astype(np.float32)


# revision 6
# speedup vs baseline: 2.6482x; 2.0993x over previous
"""Trainium2 Bass kernel for nn_MatchNet (MLP forward + 60-iter batched PDHG LP solve).

Data-parallel over 8 NeuronCores: batch 2048 -> 256 rows/core. MLP weights and
S are replicated. Each core runs the full unrolled PDHG solve on its shard.

Math (per core, batch rows b=256, n=512 structures, m=64 combos):
    Z = relu(relu(relu(X@W1+b1)@W2+b2)@W3+b3)          (computed in T layout)
    tau = sigma = 0.9/L,  alpha = tau*sigma            (L from host power iter)
    scaled duals p = tau*y1 [m,b]T, q = tau*y2, states e = x-Z, eb = xbar-Z:
      p+  = relu(p + alpha*(S@xbarT - BT))             xbar = Z + eb
      q+  = relu(q - alpha*(Z + eb))
      t1  = p+@S - q+                                  (PSUM)
      d   = e - t1 + tau
      n2  = sum_feat d^2 ; s = relu(1 - tau/max(sqrt(n2),1e-12))
      m_  = s*d ;  e+ = m_ ;  eb+ = 2*m_ - e
    out x = Z + e_final

Layouts: N-folded SBUF tiles [128, 1024]: col b*512+f = (batch 128*bt+p, feat f).
T-folded [128, 1024]: col c*256+j = (feat c*128+p, batch j).
"""

import numpy as np

N_STRUCTS = 512
N_COMBOS = 64
HID = 1024
N_ITERS = 60
N_CORES = 8
B_FULL = 2048
BC = B_FULL // N_CORES  # 256 batch rows per core
NB = BC // 128  # 2 batch sub-tiles
NF = N_STRUCTS // 128  # 4 feature chunks

_BUILD_CACHE = {}


def _power_L(S: np.ndarray) -> float:
    """Replicates reference.py's power iteration for ||K||_2 in float32."""
    S = S.astype(np.float32)
    n = S.shape[1]
    v = np.full((n,), 1.0 / np.sqrt(n), np.float32)
    for _ in range(30):
        v2 = (S.T @ (S @ v) + v).astype(np.float32)
        v = (v2 / np.float32(np.linalg.norm(v2))).astype(np.float32)
    L = np.sqrt(np.vdot(v, (S.T @ (S @ v) + v).astype(np.float32)))
    return float(L)


def _build_nc(tau: float, sigma: float):
    import contextlib

    import concourse.bacc as bacc
    import concourse.mybir as mybir
    import concourse.tile as tile

    f32 = mybir.dt.float32
    AF = mybir.ActivationFunctionType
    ALU = mybir.AluOpType
    alpha = tau * sigma

    nc = bacc.Bacc("TRN2", target_bir_lowering=False, debug=False)

    # ---- DRAM I/O (per-core shapes) ----
    d_XT = nc.dram_tensor("xt", [N_COMBOS, BC], f32, kind="ExternalInput")
    d_W1 = nc.dram_tensor("w1", [N_COMBOS, HID], f32, kind="ExternalInput")
    d_b1 = nc.dram_tensor("b1r", [128, 8], f32, kind="ExternalInput")
    d_W2 = nc.dram_tensor("w2", [HID, HID], f32, kind="ExternalInput")
    d_b2 = nc.dram_tensor("b2r", [128, 8], f32, kind="ExternalInput")
    d_W3 = nc.dram_tensor("w3", [HID, N_STRUCTS], f32, kind="ExternalInput")
    d_b3 = nc.dram_tensor("b3r", [128, 4], f32, kind="ExternalInput")
    d_S = nc.dram_tensor("s", [N_COMBOS, N_STRUCTS], f32, kind="ExternalInput")
    d_aST = nc.dram_tensor("ast", [128, 4 * N_COMBOS], f32, kind="ExternalInput")
    d_I = nc.dram_tensor("ident", [128, 128], f32, kind="ExternalInput")
    d_out = nc.dram_tensor("out", [BC, N_STRUCTS], f32, kind="ExternalOutput")

    FW = N_STRUCTS  # 512 per-b tile width

    with tile.TileContext(nc) as tc:
        stack = contextlib.ExitStack()
        with stack:
            cpool = stack.enter_context(tc.tile_pool(name="consts", bufs=1))

            def cload(dram, shape, tag):
                t = cpool.tile(shape, f32, tag=tag)
                nc.sync.dma_start(t[:], dram.ap())
                return t

            XT = cload(d_XT, [N_COMBOS, BC], "xt")
            W1 = cload(d_W1, [N_COMBOS, HID], "w1")
            b1r = cload(d_b1, [128, 8], "b1r")
            b2r = cload(d_b2, [128, 8], "b2r")
            b3r = cload(d_b3, [128, 4], "b3r")
            S_sb = cload(d_S, [N_COMBOS, N_STRUCTS], "s")
            aST = cload(d_aST, [128, 4 * N_COMBOS], "ast")
            I128 = cload(d_I, [128, 128], "ident")
            W2 = []
            for k in range(8):
                t = cpool.tile([128, HID], f32, tag=f"w2_{k}")
                nc.sync.dma_start(t[:], d_W2.ap()[k * 128 : (k + 1) * 128, :])
                W2.append(t)
            W3 = []
            for k in range(8):
                t = cpool.tile([128, N_STRUCTS], f32, tag=f"w3_{k}")
                nc.sync.dma_start(t[:], d_W3.ap()[k * 128 : (k + 1) * 128, :])
                W3.append(t)

            # ---- MLP forward, T layout ----
            zt = []  # Z^T tiles [128, BC] x4
            with (
                tc.tile_pool(name="mlp_sb", bufs=1) as mpool,
                tc.tile_pool(name="mlp_ps", bufs=4, space="PSUM") as mpsum,
            ):
                z1t = []
                for t in range(8):
                    ps = mpsum.tile([128, BC], f32, tag="mm")
                    nc.tensor.matmul(
                        ps[:], W1[:, t * 128 : (t + 1) * 128], XT[:], start=True, stop=True
                    )
                    sb = mpool.tile([128, BC], f32, tag=f"z1_{t}")
                    nc.scalar.activation(sb[:], ps[:], AF.Relu, bias=b1r[:, t : t + 1])
                    z1t.append(sb)
                z2t = []
                for t in range(8):
                    ps = mpsum.tile([128, BC], f32, tag="mm")
                    for k in range(8):
                        nc.tensor.matmul(
                            ps[:],
                            W2[k][:, t * 128 : (t + 1) * 128],
                            z1t[k][:],
                            start=(k == 0),
                            stop=(k == 7),
                        )
                    sb = mpool.tile([128, BC], f32, tag=f"z2_{t}")
                    nc.scalar.activation(sb[:], ps[:], AF.Relu, bias=b2r[:, t : t + 1])
                    z2t.append(sb)
                for c in range(NF):
                    ps = mpsum.tile([128, BC], f32, tag="mm")
                    for k in range(8):
                        nc.tensor.matmul(
                            ps[:],
                            W3[k][:, c * 128 : (c + 1) * 128],
                            z2t[k][:],
                            start=(k == 0),
                            stop=(k == 7),
                        )
                    sb = cpool.tile([128, BC], f32, tag=f"zt_{c}")
                    nc.scalar.activation(sb[:], ps[:], AF.Relu, bias=b3r[:, c : c + 1])
                    zt.append(sb)

            # ---- PDHG setup ----
            spool = stack.enter_context(tc.tile_pool(name="setup", bufs=1))
            with tc.tile_pool(name="pd_ps", bufs=1, space="PSUM") as ppool:
                # cSZB = alpha*S@Z^T - alpha*B^T   [64, BC]
                ps = ppool.tile([N_COMBOS, BC], f32, tag="py1")
                for c in range(NF):
                    nc.tensor.matmul(
                        ps[:], aST[:, c * 64 : (c + 1) * 64], zt[c][:],
                        start=(c == 0), stop=False,
                    )
                naI64 = spool.tile([N_COMBOS, N_COMBOS], f32, tag="nai64")
                nc.scalar.activation(naI64[:], I128[:64, :64], AF.Copy, scale=-alpha)
                nc.tensor.matmul(ps[:], naI64[:], XT[:], start=False, stop=True)
                cSZB = spool.tile([N_COMBOS, BC], f32, tag="cszb")
                nc.scalar.activation(cSZB[:], ps[:], AF.Copy)

                # Z per-b in N layout via PE transposes
                Z, naZ = [], []
                for b in range(NB):
                    psz = ppool.tile([128, FW], f32, tag=f"pz{b}")
                    for c in range(NF):
                        nc.tensor.transpose(
                            psz[:, c * 128 : (c + 1) * 128],
                            zt[c][:, b * 128 : (b + 1) * 128],
                            I128[:],
                        )
                    zb = spool.tile([128, FW], f32, tag=f"zn{b}")
                    nc.scalar.activation(zb[:], psz[:], AF.Copy)
                    Z.append(zb)
                    nb_ = spool.tile([128, FW], f32, tag=f"naz{b}")
                    nc.scalar.activation(nb_[:], zb[:], AF.Copy, scale=-alpha)
                    naZ.append(nb_)

            # ---- PDHG state pools ----
            em_pool = stack.enter_context(tc.tile_pool(name="em", bufs=3))
            eb_pool = stack.enter_context(tc.tile_pool(name="eb", bufs=2))
            p_pool = stack.enter_context(tc.tile_pool(name="pp", bufs=2))
            q_pool = stack.enter_context(tc.tile_pool(name="qq", bufs=2))
            sc_pool = stack.enter_context(tc.tile_pool(name="scratch", bufs=2))
            ps_T = stack.enter_context(tc.tile_pool(name="ps_T", bufs=3, space="PSUM"))
            ps_y1 = stack.enter_context(tc.tile_pool(name="ps_y1", bufs=2, space="PSUM"))
            ps_t1 = stack.enter_context(tc.tile_pool(name="ps_t1", bufs=3, space="PSUM"))

            e, eb, pc, q = [], [], [], []
            for b in range(NB):
                t = em_pool.tile([128, FW], f32, tag=f"em{b}")
                nc.scalar.activation(t[:], Z[b][:], AF.Copy, scale=-1.0)
                e.append(t)
                t = eb_pool.tile([128, FW], f32, tag=f"eb{b}")
                nc.vector.tensor_copy(t[:], e[b][:])
                eb.append(t)
                t = p_pool.tile([N_COMBOS, 128], f32, tag=f"pc{b}")
                nc.vector.tensor_copy(t[:], cSZB[:, b * 128 : (b + 1) * 128])
                pc.append(t)
                t = q_pool.tile([128, FW], f32, tag=f"q{b}")
                nc.gpsimd.memset(t[:], 0.0)
                q.append(t)

            for it in range(N_ITERS):
                for b in range(NB):
                    # 1) ebT_b = transpose(eb_b)
                    psT = ps_T.tile([128, FW], f32, tag="pT")
                    for c in range(NF):
                        nc.tensor.transpose(
                            psT[:, c * 128 : (c + 1) * 128],
                            eb[b][:, c * 128 : (c + 1) * 128],
                            I128[:],
                        )
                    ebT = sc_pool.tile([128, FW], f32, tag=f"ebT{b}")
                    nc.scalar.activation(ebT[:], psT[:], AF.Copy)

                    # 2) p+ = relu(pc + alpha*S@ebT); pc+ = p+ + cSZB_b
                    ps1 = ps_y1.tile([N_COMBOS, 128], f32, tag="py1")
                    nc.tensor.matmul(ps1[:], I128[:64, :64], pc[b][:], start=True, stop=False)
                    for c in range(NF):
                        nc.tensor.matmul(
                            ps1[:],
                            aST[:, c * 64 : (c + 1) * 64],
                            ebT[:, c * 128 : (c + 1) * 128],
                            start=False, stop=(c == NF - 1),
                        )
                    p_new = p_pool.tile([N_COMBOS, 128], f32, tag=f"p{b}")
                    nc.scalar.activation(p_new[:], ps1[:], AF.Relu)
                    pc_new = p_pool.tile([N_COMBOS, 128], f32, tag=f"pc{b}")
                    nc.vector.tensor_add(
                        pc_new[:], p_new[:], cSZB[:, b * 128 : (b + 1) * 128]
                    )

                    # 3) q+ = relu((q - alpha*Z) - alpha*eb)
                    w_ = sc_pool.tile([128, FW], f32, tag=f"w{b}")
                    nc.gpsimd.tensor_add(w_[:], q[b][:], naZ[b][:])
                    h_ = sc_pool.tile([128, FW], f32, tag=f"h{b}")
                    nc.vector.scalar_tensor_tensor(
                        h_[:], eb[b][:], -alpha, w_[:], op0=ALU.mult, op1=ALU.add
                    )
                    q_new = q_pool.tile([128, FW], f32, tag=f"q{b}")
                    nc.scalar.activation(q_new[:], h_[:], AF.Relu)

                    # 4) P1 = p+@S (PSUM); u = (e + tau) + q+
                    ps3 = ps_t1.tile([128, FW], f32, tag="pt1")
                    nc.tensor.matmul(ps3[:], p_new[:], S_sb[:], start=True, stop=True)
                    u = sc_pool.tile([128, FW], f32, tag=f"u{b}")
                    nc.gpsimd.scalar_tensor_tensor(
                        u[:], e[b][:], tau, q_new[:], op0=ALU.add, op1=ALU.add
                    )

                    # 5) d = -P1 + u
                    d = sc_pool.tile([128, FW], f32, tag=f"d{b}")
                    nc.vector.scalar_tensor_tensor(
                        d[:], ps3[:], -1.0, u[:], op0=ALU.mult, op1=ALU.add
                    )

                    # 6) n2 = sum_f d^2; 7) s = relu(1 - tau/max(sqrt(n2),1e-12))
                    n2 = sc_pool.tile([128, 1], f32, tag=f"n2{b}")
                    if b == 0:
                        dsq = sc_pool.tile([128, FW], f32, tag=f"dsq{b}")
                        nc.scalar.activation(
                            dsq[:], d[:], AF.Square, accum_out=n2[:]
                        )
                    else:
                        dsq = sc_pool.tile([128, FW], f32, tag=f"dsq{b}")
                        nc.vector.tensor_tensor_reduce(
                            dsq[:], d[:], d[:],
                            scale=1.0, scalar=0.0,
                            op0=ALU.mult, op1=ALU.add, accum_out=n2[:],
                        )
                    nmax = sc_pool.tile([128, 1], f32, tag=f"nmax{b}")
                    nc.vector.tensor_scalar_max(nmax[:], n2[:], 1e-24)
                    nrm = sc_pool.tile([128, 1], f32, tag=f"nrm{b}")
                    nc.scalar.activation(nrm[:], nmax[:], AF.Sqrt)
                    rr = sc_pool.tile([128, 1], f32, tag=f"rr{b}")
                    rs = sc_pool.tile([128, 1], f32, tag=f"rs{b}")
                    nc.vector.reciprocal_approx_accurate(rr[:], nrm[:], rs[:])
                    s = sc_pool.tile([128, 1], f32, tag=f"s{b}")
                    nc.scalar.activation(s[:], rr[:], AF.Relu, bias=1.0, scale=-tau)

                    # 8) m = s (.) d -> e+
                    m_ = em_pool.tile([128, FW], f32, tag=f"em{b}")
                    if b == 0:
                        nc.vector.tensor_scalar_mul(m_[:], d[:], s[:])
                    else:
                        nc.scalar.activation(m_[:], d[:], AF.Copy, scale=s[:])

                    # 9) eb+ = 2m - e
                    eb_new = eb_pool.tile([128, FW], f32, tag=f"eb{b}")
                    nc.vector.ln_bwd_dx(
                        eb_new[:], m_[:], e[b][:], mean_dyx=0.5, mean_dy=0.0, scale=2.0
                    )

                    e[b], eb[b], q[b], pc[b] = m_, eb_new, q_new, pc_new

            # ---- output: x = Z + e ----
            for b in range(NB):
                xout = sc_pool.tile([128, FW], f32, tag=f"xout{b}")
                nc.vector.tensor_add(xout[:], Z[b][:], e[b][:])
                nc.sync.dma_start(d_out.ap()[b * 128 : (b + 1) * 128, :], xout[:])

    nc.finalize()
    return nc


def _get_nc(S: np.ndarray):
    key = hash(S.tobytes())
    if key not in _BUILD_CACHE:
        L = _power_L(S)
        tau = 0.9 / L
        sigma = 0.9 / L
        _BUILD_CACHE[key] = (_build_nc(tau, sigma), tau, sigma)
    return _BUILD_CACHE[key]


def _make_in_maps(X, W1, b1, W2, b2, W3, b3, S, tau, sigma):
    alpha = np.float32(tau * sigma)
    Xflat = np.ascontiguousarray(X.reshape(B_FULL, N_COMBOS)).astype(np.float32)
    # aST packed: alpha * S.T chunks [128, 64] side by side -> [128, 256]
    aST_full = (alpha * S.T).astype(np.float32)  # [512, 64]
    aST = np.concatenate(
        [aST_full[c * 128 : (c + 1) * 128, :] for c in range(NF)], axis=1
    )
    aST = np.ascontiguousarray(aST)
    b1r = np.ascontiguousarray(b1.reshape(8, 128).T).astype(np.float32)
    b2r = np.ascontiguousarray(b2.reshape(8, 128).T).astype(np.float32)
    b3r = np.ascontiguousarray(b3.reshape(4, 128).T).astype(np.float32)
    I128 = np.eye(128, dtype=np.float32)
    nI128 = (-np.eye(128)).astype(np.float32)
    naI128 = (-alpha * np.eye(128)).astype(np.float32)
    shared = {
        "w1": np.ascontiguousarray(W1.astype(np.float32)),
        "b1r": b1r,
        "w2": np.ascontiguousarray(W2.astype(np.float32)),
        "b2r": b2r,
        "w3": np.ascontiguousarray(W3.astype(np.float32)),
        "b3r": b3r,
        "s": np.ascontiguousarray(S.astype(np.float32)),
        "ast": aST,
        "ident": I128,
        "nident": nI128,
        "naident": naI128,
    }
    in_maps = []
    for c in range(N_CORES):
        xt = np.ascontiguousarray(Xflat[c * BC : (c + 1) * BC, :].T)
        in_maps.append({**shared, "xt": xt})
    return in_maps


def kernel(X, W1, b1, W2, b2, W3, b3, S, batch_size):
    from concourse.bass_utils import run_bass_kernel_spmd

    X = np.asarray(X)
    S = np.asarray(S)
    nc, tau, sigma = _get_nc(np.ascontiguousarray(S.astype(np.float32)))
    in_maps = _make_in_maps(
        X,
        np.asarray(W1),
        np.asarray(b1),
        np.asarray(W2),
        np.asarray(b2),
        np.asarray(W3),
        np.asarray(b3),
        S,
        tau,
        sigma,
    )
    res = run_bass_kernel_spmd(nc, in_maps, core_ids=list(range(N_CORES)))
    out = np.concatenate([res.results[c]["out"] for c in range(N_CORES)], axis=0)
    return out.astype(np.float32)
